# revision 1
# baseline (speedup 1.0000x reference)
"""ConformerDecoder Trainium2 Bass kernel (stripe-pipelined).

Sharding: pure data-parallel over batch B=8 -> one sample per NeuronCore.

Activations live transposed [feature-on-partitions, T-free]; all matmuls use
natural-layout bf16 weights as lhsT with fp32 PSUM accumulation.

Key structure vs the v1 kernel:
- Every LayerNorm is stripe-pipelined: T is split in two 512-token stripes;
  a phase's stripe-1 matmuls run on PE while stripe-0's LN stats (ACT squares
  -> PE ones-matmul sums -> DVE var/Newton-rsqrt chain) and normalize run on
  ACT/DVE.  Stat matmuls are injected mid-loop into the neighboring matmul
  stretch so PE never drains.
- Attention computes scores TRANSPOSED (k-on-partitions) with 3 chunk
  matmuls per (head-pair, q-block, head): exp on ACT, band-mask multiply on
  DVE, then AV + softmax-denominator ones-matmuls accumulate straight off the
  masked exp tile -- no PE transposes, no per-row softmax scaling; one
  reciprocal + one scaled PSUM evacuation per (head-pair, q-block).
- Conv GroupNorm sum/sumsq matmuls accumulate inside the depthwise-conv loop;
  GN affine + SiLU fuse into a single ACT op per channel chunk feeding
  m-outer pw2 accumulation.
- DVE relief: squares and PSUM evacuations on ACT, GLU via Tanh, dw-conv
  diag strips built with one broadcast-view multiply per chunk.
"""

import os
import sys
from contextlib import ExitStack

for _p in ("/opt/trn_rl_repo",):
    if _p not in sys.path:
        sys.path.insert(0, _p)

import numpy as np
import ml_dtypes

import concourse.bass as bass
import concourse.tile as tile
from concourse import bacc
from concourse import mybir
from concourse.bass_utils import run_bass_kernel_spmd

BF16 = mybir.dt.bfloat16
F32 = mybir.dt.float32
AF = mybir.ActivationFunctionType
OP = mybir.AluOpType

L, D, H, T, B = 4, 512, 8, 1024, 8
FF = 4 * D            # 2048
EC = 2 * D            # 1024 conv channels
KK = 31               # conv kernel size
WIN = 64              # attention window
DH = D // H           # 64
P = 128
DC = D // P           # 4 feature chunks
FC = FF // P          # 16
CC = EC // P          # 8
TB = T // P           # 8 token blocks
NT = 512              # matmul moving free dim
TC = T // NT          # 2 t-stripes
KW = 3 * P            # 384: attention window width
EPS = 1e-5

TRACE = False          # set by test.py for profiling runs
TRACE_KW = {}
LAST_RESULT = None     # BassKernelResults of last run (read by test.py)
LAYERS = int(os.environ.get("CONF_LAYERS", str(L)))

# bias row indices in the packed bias tensor
BR_F1B1, BR_F1B2, BR_Q, BR_K, BR_V, BR_O, BR_P1, BR_P2, BR_F2B1, BR_F2B2 = range(10)


def _band_masks_t():
    """Transposed band masks [P, 3, P] bf16: maskT[k, c, q] = valid(q, c*128+k).

    Window column c128+k for q-block qb is absolute key kk = qb*128 - 128 +
    (c*128+k); row q is query qb*128 + q.  Valid iff |q-kk| <= WIN/2 and
    0 <= kk < T.
    """
    q = np.arange(P)[:, None]
    c = np.arange(KW)[None, :]
    band = np.abs(q + P - c) <= WIN // 2          # [q, kw]
    q0 = band & (c >= P)
    q7 = band & (c <= 255)

    def to(m):
        mt = m.T.reshape(3, P, P).transpose(1, 0, 2)   # [k, c, q]
        return np.ascontiguousarray(mt.astype(ml_dtypes.bfloat16))

    return to(band), to(q0), to(q7)


def build_program(flags):
    ln_gen = not flags["ln_trivial"]
    bias_gen = not flags["bias_trivial"]
    fin_gen = not flags["final_trivial"]
    dwb_gen = not flags["dwb_trivial"]

    nc = bacc.Bacc("TRN2", target_bir_lowering=False, debug=False)

    xt_d = nc.dram_tensor("x_t", [D, T], F32, kind="ExternalInput").ap()
    out_d = nc.dram_tensor("out_t", [D, T], F32, kind="ExternalOutput").ap()

    def win(name, shape):
        return nc.dram_tensor(name, shape, BF16, kind="ExternalInput").ap()

    w_f1a = win("f1w1", [L, D, FF])
    w_f1b = win("f1w2", [L, FF, D])
    w_f2a = win("f2w1", [L, D, FF])
    w_f2b = win("f2w2", [L, FF, D])
    w_q = win("wq", [L, D, D])
    w_kk = win("wk", [L, D, D])
    w_v = win("wv", [L, D, D])
    w_o = win("wo", [L, D, D])
    w_p1 = win("pw1", [L, D, 2 * EC])
    w_p2 = win("pw2", [L, EC, D])
    w_dw = nc.dram_tensor("dw", [L, P, CC, KK], F32, kind="ExternalInput").ap()
    w_gn = nc.dram_tensor("gn_aff", [L, 2, EC], F32, kind="ExternalInput").ap()
    w_gains = (nc.dram_tensor("ln_gains", [L, 10, D], F32, kind="ExternalInput").ap()
                if ln_gen else None)
    w_fin = (nc.dram_tensor("final_aff", [2, D], F32, kind="ExternalInput").ap()
             if fin_gen else None)
    w_bias = win("biases", [L, 10, 2 * EC]) if bias_gen else None
    w_dwb = (nc.dram_tensor("dwb", [L, P, CC], F32, kind="ExternalInput").ap()
             if dwb_gen else None)

    mt_mid, mt_q0, mt_q7 = _band_masks_t()
    bmid_d = nc.inline_tensor(mt_mid, "bandt_mid").ap()
    bq0_d = nc.inline_tensor(mt_q0, "bandt_q0").ap()
    bq7_d = nc.inline_tensor(mt_q7, "bandt_q7").ap()
    id_d = nc.inline_tensor(np.eye(P, dtype=ml_dtypes.bfloat16), "ident").ap()
    ones_d = nc.inline_tensor(np.ones((P, NT), dtype=ml_dtypes.bfloat16), "ones").ap()

    with tile.TileContext(nc) as tc, ExitStack() as ctx:
        pers = ctx.enter_context(tc.tile_pool(name="pers", bufs=1))
        wpool = ctx.enter_context(tc.tile_pool(name="w", bufs=1))
        spool = ctx.enter_context(tc.tile_pool(name="stat", bufs=1))
        hpool = ctx.enter_context(tc.tile_pool(name="h1", bufs=3))
        tpool = ctx.enter_context(tc.tile_pool(name="tanh", bufs=3))
        atpool = ctx.enter_context(tc.tile_pool(name="attn", bufs=1))
        smpool = ctx.enter_context(tc.tile_pool(name="small", bufs=8))
        cpool = ctx.enter_context(tc.tile_pool(name="conv", bufs=1))
        opool = ctx.enter_context(tc.tile_pool(name="outp", bufs=2))
        psum = ctx.enter_context(tc.tile_pool(name="ps", bufs=8, space="PSUM"))

        ident = pers.tile([P, P], BF16, tag="ident")
        nc.sync.dma_start(ident, id_d)
        ones = pers.tile([P, NT], BF16, tag="ones")
        nc.sync.dma_start(ones, ones_d)
        bands = {}
        for nm, dd in (("mid", bmid_d), ("q0", bq0_d), ("q7", bq7_d)):
            bt = pers.tile([P, 3, P], BF16, tag=f"band_{nm}")
            nc.sync.dma_start(bt, dd)
            bands[nm] = bt

        x = pers.tile([P, DC, T], BF16, tag="x")
        with tc.tile_pool(name="xin", bufs=2) as xinp:
            for kc in range(DC):
                xf = xinp.tile([P, T], F32, name=f"xf{kc}", tag="xf")
                nc.sync.dma_start(
                    xf, xt_d.rearrange("(c p) t -> c p t", p=P)[kc])
                nc.scalar.copy(out=x[:, kc], in_=xf)

        xh = pers.tile([P, DC, T], BF16, tag="xhat")
        qT = pers.tile([P, DC, T], BF16, tag="qT")
        kT = pers.tile([P, DC, T + 2 * P], BF16, tag="kT")
        vN = pers.tile([P, TB + 2, D], BF16, tag="vN")
        oT = pers.tile([P, DC, T], BF16, tag="oT")
        c2 = pers.tile([P, CC, T], BF16, tag="c2")
        nc.vector.memset(kT[:, :, 0:P], 0.0)
        nc.vector.memset(kT[:, :, P + T :], 0.0)
        nc.vector.memset(vN[:, 0, :], 0.0)
        nc.vector.memset(vN[:, TB + 1, :], 0.0)

        # ---------------- LayerNorm (stripe-pipelined) ----------------
        # state per LN instance: x2, psum pair, r_bf/mr_bf per stripe

        def ln_rstd(var, out_r, niter=2):
            r = spool.tile(list(var.shape), F32, tag="newt_r", bufs=2)
            nc.vector.tensor_scalar(
                out=r, in0=var, scalar1=-0.5, scalar2=1.5,
                op0=OP.mult, op1=OP.add)
            t1 = spool.tile(list(var.shape), F32, tag="newt_t", bufs=2)
            for it in range(niter):
                nc.vector.tensor_tensor(t1, r, r, OP.mult)
                nc.vector.scalar_tensor_tensor(
                    out=t1, in0=t1, scalar=-0.5, in1=var, op0=OP.mult, op1=OP.mult)
                dst = out_r if it == niter - 1 else r
                nc.vector.scalar_tensor_tensor(
                    out=dst, in0=t1, scalar=1.5, in1=r, op0=OP.add, op1=OP.mult)

        class LNState:
            __slots__ = ("src", "lidx", "which", "ps", "r_bf", "mr_bf", "x2")

        def ln_sq(src, lidx, which, tci, st=None):
            """ACT squares for stripe tci; allocates state on first call."""
            if st is None:
                st = LNState()
                st.src, st.lidx, st.which = src, lidx, which
                st.ps = [None, None]
                st.r_bf = spool.tile([P, T], BF16, tag="r_bf", bufs=3)
                st.mr_bf = spool.tile([P, T], BF16, tag="mr_bf", bufs=3)
                st.x2 = [None, None]
            sl = slice(tci * NT, (tci + 1) * NT)
            x2 = spool.tile([P, DC, NT], BF16, tag="ln_x2", bufs=1)
            nc.scalar.activation(out=x2, in_=st.src[:, :, sl], func=AF.Square)
            st.x2[tci] = x2
            return st

        def ln_sums(st, tci):
            """PE ones-matmul sums for stripe tci (inject into a mm stretch)."""
            sl = slice(tci * NT, (tci + 1) * NT)
            ps_s = psum.tile([P, NT], F32, tag="mm", name="ps_s")
            ps_q = psum.tile([P, NT], F32, tag="mm", name="ps_q")
            for kc in range(DC):
                nc.tensor.matmul(ps_s, lhsT=ones[:, 0:P], rhs=st.src[:, kc, sl],
                                 start=(kc == 0), stop=(kc == DC - 1))
            for kc in range(DC):
                nc.tensor.matmul(ps_q, lhsT=ones[:, 0:P], rhs=st.x2[tci][:, kc],
                                 start=(kc == 0), stop=(kc == DC - 1))
            st.ps[tci] = (ps_s, ps_q)

        def ln_var(st, tci):
            """msq/mean on ACT, var + Newton rsqrt + mr on DVE."""
            sl = slice(tci * NT, (tci + 1) * NT)
            ps_s, ps_q = st.ps[tci]
            msq = spool.tile([P, NT], F32, tag="msq", bufs=1)
            nc.scalar.activation(out=msq, in_=ps_s, func=AF.Square, scale=1.0 / D)
            mean = spool.tile([P, NT], F32, tag="mean", bufs=1)
            nc.scalar.mul(out=mean, in_=ps_s, mul=1.0 / D)
            var = spool.tile([P, NT], F32, tag="var", bufs=2)
            nc.vector.scalar_tensor_tensor(
                out=var, in0=ps_q, scalar=1.0 / D, in1=msq,
                op0=OP.mult, op1=OP.subtract)
            ln_rstd(var, st.r_bf[:, sl])
            nc.vector.tensor_tensor(st.mr_bf[:, sl], mean, st.r_bf[:, sl], OP.mult)

        def ln_norm(st, tci, dst, out_stream=None, fin_sb=None):
            """normalize stripe tci of src into dst (DVE)."""
            sl = slice(tci * NT, (tci + 1) * NT)
            g_sb = None
            if w_gains is not None:
                g_sb = spool.tile([P, 2, DC], F32, tag="g_sb", bufs=2)
                nc.sync.dma_start(
                    g_sb, w_gains[st.lidx, 2 * st.which : 2 * st.which + 2]
                    .rearrange("g (c p) -> p g c", p=P))
            for kc in range(DC):
                u = tpool.tile([P, NT], BF16, tag="ln_u")
                nc.vector.tensor_tensor(u, st.src[:, kc, sl], st.r_bf[:, sl],
                                        OP.mult)
                if dst is not None:
                    tgt = dst[:, kc, sl]
                    nc.vector.tensor_tensor(tgt, u, st.mr_bf[:, sl], OP.subtract)
                    if g_sb is not None:
                        nc.scalar.activation(
                            out=tgt, in_=tgt, func=AF.Identity,
                            bias=g_sb[:, 1, kc : kc + 1], scale=g_sb[:, 0, kc : kc + 1])
                if out_stream is not None:
                    dview = out_stream
                    of = opool.tile([P, NT], F32, tag="of")
                    nc.vector.tensor_tensor(of, u, st.mr_bf[:, sl], OP.subtract)
                    if g_sb is not None:
                        nc.scalar.activation(
                            out=of, in_=of, func=AF.Identity,
                            bias=g_sb[:, 1, kc : kc + 1], scale=g_sb[:, 0, kc : kc + 1])
                    if fin_sb is not None:
                        nc.scalar.activation(
                            out=of, in_=of, func=AF.Identity,
                            bias=fin_sb[:, 1, kc : kc + 1], scale=fin_sb[:, 0, kc : kc + 1])
                    nc.sync.dma_start(dview[:, kc, sl], of)

        def load_w(dram, lidx, tag):
            _, fin, fout = dram.shape
            wt = wpool.tile([P, fin // P, fout], BF16, tag=tag)
            nc.sync.dma_start(wt, dram[lidx].rearrange("(c p) f -> p c f", p=P))
            return wt

        bias_sb = [None]

        def bias_mm(ps, row, mslice, tcslice_n):
            if bias_sb[0] is None:
                return
            nc.tensor.matmul(
                ps, lhsT=bias_sb[0][0:1, row, mslice], rhs=ones[0:1, 0:tcslice_n],
                start=False, stop=True, skip_group_check=True)

        def run_inject(inject, idx):
            if inject and idx in inject:
                for fn in inject[idx]:
                    fn()

        def emit_ffn_stripe(w1, w2, rows, src, tci, inject=None):
            """One stripe of an FFN: mms + silu + residual adds into x."""
            sl = slice(tci * NT, (tci + 1) * NT)
            acc = [psum.tile([P, NT], F32, tag="mm", name=f"acc{i}")
                   for i in range(DC)]
            for m in range(FC):
                run_inject(inject, m)
                ph = psum.tile([P, NT], F32, tag="mm", name="ph")
                for kc in range(DC):
                    nc.tensor.matmul(
                        ph, lhsT=w1[:, kc, m * P : (m + 1) * P], rhs=src[:, kc, sl],
                        start=(kc == 0), stop=(kc == DC - 1 and not bias_gen))
                bias_mm(ph, rows[0], slice(m * P, (m + 1) * P), NT)
                hb = hpool.tile([P, NT], BF16, tag="h1")
                nc.scalar.activation(out=hb, in_=ph, func=AF.Silu)
                for dcc in range(DC):
                    nc.tensor.matmul(
                        acc[dcc], lhsT=w2[:, m, dcc * P : (dcc + 1) * P], rhs=hb,
                        start=(m == 0), stop=(m == FC - 1 and not bias_gen),
                        skip_group_check=True)
            run_inject(inject, FC)
            for dcc in range(DC):
                bias_mm(acc[dcc], rows[1], slice(dcc * P, (dcc + 1) * P), NT)
                nc.vector.scalar_tensor_tensor(
                    out=x[:, dcc, sl], in0=acc[dcc], scalar=1.0,
                    in1=x[:, dcc, sl], op0=OP.bypass, op1=OP.add)

        # ---------------- attention sub-emitters ----------------

        def emit_qk_stripe(wq, wk, tci, inject=None):
            sl = slice(tci * NT, (tci + 1) * NT)
            for m in range(DC):
                run_inject(inject, m)
                pq = psum.tile([P, NT], F32, tag="mm", name="pq")
                for kc in range(DC):
                    nc.tensor.matmul(
                        pq, lhsT=wq[:, kc, m * P : (m + 1) * P], rhs=xh[:, kc, sl],
                        start=(kc == 0), stop=(kc == DC - 1 and not bias_gen))
                bias_mm(pq, BR_Q, slice(m * P, (m + 1) * P), NT)
                nc.scalar.copy(out=qT[:, m, sl], in_=pq)
                pk = psum.tile([P, NT], F32, tag="mm", name="pk")
                for kc in range(DC):
                    nc.tensor.matmul(
                        pk, lhsT=wk[:, kc, m * P : (m + 1) * P], rhs=xh[:, kc, sl],
                        start=(kc == 0), stop=(kc == DC - 1 and not bias_gen))
                bias_mm(pk, BR_K, slice(m * P, (m + 1) * P), NT)
                nc.scalar.copy(
                    out=kT[:, m, P + tci * NT : P + (tci + 1) * NT], in_=pk)

        def emit_v_blocks(wv, tbs, inject=None):
            for i, tb in enumerate(tbs):
                run_inject(inject, i)
                pv = psum.tile([P, NT], F32, tag="mm", name="pv")
                for kc in range(DC):
                    nc.tensor.matmul(
                        pv, lhsT=xh[:, kc, tb * P : (tb + 1) * P],
                        rhs=wv[:, kc, 0:D],
                        start=(kc == 0), stop=(kc == DC - 1 and not bias_gen))
                if bias_gen:
                    nc.tensor.matmul(
                        pv, lhsT=ones[0:1, 0:P], rhs=bias_sb[0][0:1, BR_V, 0:D],
                        start=False, stop=True, skip_group_check=True)
                nc.scalar.copy(out=vN[:, tb + 1, :], in_=pv)

        def emit_attn_core(qb, inject=None):
            band = bands["q0"] if qb == 0 else (
                bands["q7"] if qb == TB - 1 else bands["mid"])
            for hp in range(DC):
                run_inject(inject, hp)
                po = psum.tile([P, P], F32, tag="mm", name="po")
                den = psum.tile([P, P], F32, tag="mm", name="den")
                for hh in range(2):
                    pr = slice(hh * DH, (hh + 1) * DH)
                    st = psum.tile([P, 3, P], F32, tag="mm", name="st")
                    for c in range(3):
                        nc.tensor.matmul(
                            st[:, c, :],
                            lhsT=kT[pr, hp, qb * P + c * P : qb * P + (c + 1) * P],
                            rhs=qT[pr, hp, qb * P : (qb + 1) * P],
                            start=True, stop=True, skip_group_check=True)
                    at = atpool.tile([P, 3, P], BF16, tag="at", bufs=3)
                    nc.scalar.activation(out=at, in_=st, func=AF.Exp)
                    nc.vector.tensor_tensor(at, at, band, OP.mult)
                    h = hp * 2 + hh
                    hc = slice(h * DH, (h + 1) * DH)
                    for c in range(3):
                        nc.tensor.matmul(
                            po[pr, :], lhsT=vN[:, qb + c, hc], rhs=at[:, c, :],
                            start=(c == 0), stop=(c == 2), skip_group_check=True)
                    for c in range(3):
                        nc.tensor.matmul(
                            den[pr, :], lhsT=ones[:, 0:DH], rhs=at[:, c, :],
                            start=(c == 0), stop=(c == 2), skip_group_check=True)
                rden = atpool.tile([P, P], F32, tag="rden", bufs=2)
                nc.vector.reciprocal(out=rden, in_=den)
                nc.vector.tensor_tensor(
                    oT[:, hp, qb * P : (qb + 1) * P], po, rden, OP.mult)

        def emit_outproj_stripe(wo, tci, inject=None):
            sl = slice(tci * NT, (tci + 1) * NT)
            for m in range(DC):
                run_inject(inject, m)
                pp = psum.tile([P, NT], F32, tag="mm", name="pp")
                for kc in range(DC):
                    nc.tensor.matmul(
                        pp, lhsT=wo[:, kc, m * P : (m + 1) * P], rhs=oT[:, kc, sl],
                        start=(kc == 0), stop=(kc == DC - 1 and not bias_gen))
                bias_mm(pp, BR_O, slice(m * P, (m + 1) * P), NT)
                nc.vector.scalar_tensor_tensor(
                    out=x[:, m, sl], in0=pp, scalar=1.0, in1=x[:, m, sl],
                    op0=OP.bypass, op1=OP.add)

        # ---------------- conv sub-emitters ----------------

        def emit_conv(l, p1, p2, dwt16, dwb_sb, gn_ps, inject=None):
            """pw1+GLU+dwconv with GN sums accumulated in-loop."""
            gs, gq = gn_ps
            for m in range(CC):
                run_inject(inject, m)
                cp = cpool.tile([P, KK - 1 + T + 1], BF16, tag="cp", bufs=2)
                co = cpool.tile([P, KK - 1 + T + 1], BF16, tag="co", bufs=2)
                nc.vector.memset(cp[:, 0 : KK // 2], 0.0)
                nc.vector.memset(cp[:, KK // 2 + T :], 0.0)
                # diag strips for this channel chunk: one broadcast multiply
                strip = cpool.tile([P, KK, P], BF16, tag="strip", bufs=2)
                nc.vector.tensor_tensor(
                    strip,
                    ident.unsqueeze(1).to_broadcast((P, KK, P)),
                    dwt16[:, m, :].unsqueeze(2).to_broadcast((P, KK, P)),
                    OP.mult)
                for tci in range(TC):
                    sl = slice(tci * NT, (tci + 1) * NT)
                    pb = psum.tile([P, NT], F32, tag="mm", name="pb")
                    for kc in range(DC):
                        nc.tensor.matmul(
                            pb, lhsT=p1[:, kc, EC + m * P : EC + (m + 1) * P],
                            rhs=xh[:, kc, sl],
                            start=(kc == 0), stop=(kc == DC - 1 and not bias_gen))
                    bias_mm(pb, BR_P1, slice(EC + m * P, EC + (m + 1) * P), NT)
                    tb_ = tpool.tile([P, NT], BF16, tag="th")
                    nc.scalar.activation(out=tb_, in_=pb, func=AF.Tanh, scale=0.5)
                    pa = psum.tile([P, NT], F32, tag="mm", name="pa")
                    for kc in range(DC):
                        nc.tensor.matmul(
                            pa, lhsT=p1[:, kc, m * P : (m + 1) * P],
                            rhs=xh[:, kc, sl],
                            start=(kc == 0), stop=(kc == DC - 1 and not bias_gen))
                    bias_mm(pa, BR_P1, slice(m * P, (m + 1) * P), NT)
                    nc.vector.scalar_tensor_tensor(
                        out=cp[:, KK // 2 + tci * NT : KK // 2 + (tci + 1) * NT],
                        in0=tb_, scalar=1.0, in1=pa, op0=OP.add, op1=OP.mult)
                nc.vector.tensor_copy(out=co[:, 0 : KK - 1 + T],
                                      in_=cp[:, 1 : KK + T])
                for tci in range(TC):
                    pc = psum.tile([P, NT], F32, tag="mm", name="pc")
                    for kk in range(KK):
                        rhs = (cp[:, kk + tci * NT : kk + tci * NT + NT]
                               if kk % 2 == 0 else
                               co[:, kk - 1 + tci * NT : kk - 1 + tci * NT + NT])
                        nc.tensor.matmul(pc, lhsT=strip[:, kk, :], rhs=rhs,
                                         start=(kk == 0), stop=(kk == KK - 1),
                                         skip_group_check=True)
                    csl = c2[:, m, tci * NT : (tci + 1) * NT]
                    if dwb_sb is not None:
                        nc.scalar.activation(out=csl, in_=pc, func=AF.Identity,
                                             bias=dwb_sb[:, m : m + 1])
                    else:
                        nc.scalar.copy(out=csl, in_=pc)
                # GN sums for this chunk (accumulate across m and tci)
                c2sq = spool.tile([P, T], BF16, tag="gnsq", bufs=2)
                nc.scalar.activation(out=c2sq, in_=c2[:, m], func=AF.Square)
                for tci in range(TC):
                    sl = slice(tci * NT, (tci + 1) * NT)
                    nc.tensor.matmul(gs, lhsT=ones[:, 0:P], rhs=c2[:, m, sl],
                                     start=(m == 0 and tci == 0),
                                     stop=(m == CC - 1 and tci == TC - 1),
                                     skip_group_check=True)
                    nc.tensor.matmul(gq, lhsT=ones[:, 0:P], rhs=c2sq[:, sl],
                                     start=(m == 0 and tci == 0),
                                     stop=(m == CC - 1 and tci == TC - 1),
                                     skip_group_check=True)

        def emit_gn_finalize(l, gn_ps):
            """GroupNorm scalar chain -> per-channel affine (a_t, b_t)."""
            gs_ps, gq_ps = gn_ps
            rs = smpool.tile([P, 1], F32, tag="gs")
            rq = smpool.tile([P, 1], F32, tag="gq")
            nc.vector.tensor_reduce(out=rs, in_=gs_ps,
                                    axis=mybir.AxisListType.X, op=OP.add)
            nc.vector.tensor_reduce(out=rq, in_=gq_ps,
                                    axis=mybir.AxisListType.X, op=OP.add)
            mg = smpool.tile([P, 1], F32, tag="mg")
            nc.vector.tensor_scalar_mul(out=mg, in0=rs, scalar1=1.0 / (EC * T))
            msqg = smpool.tile([P, 1], F32, tag="msqg")
            nc.vector.tensor_tensor(msqg, mg, mg, OP.mult)
            varg = smpool.tile([P, 1], F32, tag="varg")
            nc.vector.scalar_tensor_tensor(
                out=varg, in0=rq, scalar=1.0 / (EC * T), in1=msqg,
                op0=OP.mult, op1=OP.subtract)
            nc.vector.tensor_scalar_add(out=varg, in0=varg, scalar1=EPS)
            rg = smpool.tile([P, 1], F32, tag="rg")
            ln_rstd(varg, rg, niter=14)
            gaff = spool.tile([P, 2, CC], F32, tag="gaff", bufs=2)
            nc.sync.dma_start(gaff, w_gn[l].rearrange("g (c p) -> p g c", p=P))
            a_t = spool.tile([P, CC], F32, tag="a_t", bufs=2)
            nc.vector.tensor_scalar_mul(out=a_t, in0=gaff[:, 0], scalar1=rg)
            mneg = smpool.tile([P, 1], F32, tag="mneg")
            nc.vector.tensor_scalar_mul(out=mneg, in0=mg, scalar1=-1.0)
            b_t = spool.tile([P, CC], F32, tag="b_t", bufs=2)
            nc.vector.scalar_tensor_tensor(
                out=b_t, in0=a_t, scalar=mneg, in1=gaff[:, 1],
                op0=OP.mult, op1=OP.add)
            return a_t, b_t

        def emit_pw2(p2, a_t, b_t, tci, inject=None):
            """GN affine + SiLU fused on ACT per chunk; m-outer pw2 acc."""
            sl = slice(tci * NT, (tci + 1) * NT)
            acc = [psum.tile([P, NT], F32, tag="mm", name=f"cacc{i}")
                   for i in range(DC)]
            for m in range(CC):
                run_inject(inject, m)
                if tci == 0:
                    nc.scalar.activation(
                        out=c2[:, m], in_=c2[:, m], func=AF.Silu,
                        bias=b_t[:, m : m + 1], scale=a_t[:, m : m + 1])
                for dcc in range(DC):
                    nc.tensor.matmul(
                        acc[dcc], lhsT=p2[:, m, dcc * P : (dcc + 1) * P],
                        rhs=c2[:, m, sl],
                        start=(m == 0), stop=(m == CC - 1 and not bias_gen),
                        skip_group_check=True)
            run_inject(inject, CC)
            for dcc in range(DC):
                bias_mm(acc[dcc], BR_P2, slice(dcc * P, (dcc + 1) * P), NT)
                nc.vector.scalar_tensor_tensor(
                    out=x[:, dcc, sl], in0=acc[dcc], scalar=1.0,
                    in1=x[:, dcc, sl], op0=OP.bypass, op1=OP.add)

        # ================= layer driver =================

        # initial LN for layer 0 (x raw -> xh)
        st0 = ln_sq(x, 0, 0, 0)
        ln_sums(st0, 0)
        ln_var(st0, 0)
        ln_sq(x, 0, 0, 1, st0)
        ln_sums(st0, 1)
        ln_var(st0, 1)
        ln_norm(st0, 0, xh)
        ln_norm(st0, 1, xh)

        # pending LN-norm emitters carried across phases
        for l in range(LAYERS):
            if bias_gen:
                bt = wpool.tile([1, 10, 2 * EC], BF16, tag="bias")
                nc.sync.dma_start(bt, w_bias[l])
                bias_sb[0] = bt

            w1 = load_w(w_f1a, l, "w1")
            w2 = load_w(w_f1b, l, "w2")
            src1 = xh if l == 0 else x

            # ===== FFN1 (+ attn-LN pipelined) =====
            stA = [None]
            emit_ffn_stripe(w1, w2, (BR_F1B1, BR_F1B2), src1, 0)
            stA[0] = ln_sq(x, l, 1, 0)
            emit_ffn_stripe(
                w1, w2, (BR_F1B1, BR_F1B2), src1, 1,
                inject={4: [lambda: ln_sums(stA[0], 0)],
                        6: [lambda: ln_var(stA[0], 0)],
                        10: [lambda: ln_norm(stA[0], 0, xh)]})
            ln_sq(x, l, 1, 1, stA[0])

            # ===== attention =====
            wq = load_w(w_q, l, "wq")
            wk = load_w(w_kk, l, "wk")
            wv = load_w(w_v, l, "wv")
            wo = load_w(w_o, l, "wo")
            emit_qk_stripe(wq, wk, 0,
                           inject={1: [lambda: ln_sums(stA[0], 1)],
                                   2: [lambda: ln_var(stA[0], 1)],
                                   3: [lambda: ln_norm(stA[0], 1, xh)]})
            emit_qk_stripe(wq, wk, 1)
            emit_v_blocks(wv, range(TB))
            stC = [None]

            def core_tail_0():
                emit_outproj_stripe(wo, 0)
                stC[0] = ln_sq(x, l, 2, 0)

            for qb in range(4):
                emit_attn_core(qb)
            core_tail_0()
            emit_attn_core(4, inject={2: [lambda: ln_sums(stC[0], 0)]})
            for qb in range(5, TB):
                emit_attn_core(qb)
            emit_outproj_stripe(wo, 1, inject={2: [lambda: ln_var(stC[0], 0)]})
            ln_sq(x, l, 2, 1, stC[0])
            ln_norm(stC[0], 0, xh)

            # ===== conv module =====
            p1 = load_w(w_p1, l, "w1")
            p2 = load_w(w_p2, l, "w2")
            dwt = wpool.tile([P, CC, KK], F32, tag="dw")
            nc.sync.dma_start(dwt, w_dw[l])
            dwt16 = wpool.tile([P, CC, KK], BF16, tag="dw16")
            nc.vector.tensor_copy(out=dwt16, in_=dwt)
            dwb_sb = None
            if dwb_gen:
                dwb_sb = wpool.tile([P, CC], F32, tag="dwb")
                nc.sync.dma_start(dwb_sb, w_dwb[l])
            gs = psum.tile([P, NT], F32, tag="mm", name="gn_s")
            gq = psum.tile([P, NT], F32, tag="mm", name="gn_q")

            # conv-LN stripe-1 stats emitted just before the pw1/dw stretch:
            # pw1 m=0 reads xh stripe 1, so its normalize must precede it
            def conv_inject():
                ln_sums(stC[0], 1)
                ln_var(stC[0], 1)
                ln_norm(stC[0], 1, xh)

            emit_conv(l, p1, p2, dwt16, dwb_sb, (gs, gq),
                      inject={0: [conv_inject]})

            a_t, b_t = emit_gn_finalize(l, (gs, gq))
            st2 = [None]
            emit_pw2(p2, a_t, b_t, 0)
            st2[0] = ln_sq(x, l, 3, 0)
            emit_pw2(p2, a_t, b_t, 1,
                     inject={2: [lambda: ln_sums(st2[0], 0)],
                             4: [lambda: ln_var(st2[0], 0)],
                             6: [lambda: ln_norm(st2[0], 0, xh)]})
            ln_sq(x, l, 3, 1, st2[0])

            # ===== FFN2 (+ blk-LN pipelined) =====
            w1b = load_w(w_f2a, l, "w1")
            w2b = load_w(w_f2b, l, "w2")
            stB = [None]
            emit_ffn_stripe(
                w1b, w2b, (BR_F2B1, BR_F2B2), xh, 0,
                inject={2: [lambda: ln_sums(st2[0], 1)],
                        4: [lambda: ln_var(st2[0], 1)],
                        8: [lambda: ln_norm(st2[0], 1, xh)]})
            stB[0] = ln_sq(x, l, 4, 0)
            emit_ffn_stripe(
                w1b, w2b, (BR_F2B1, BR_F2B2), xh, 1,
                inject={4: [lambda: ln_sums(stB[0], 0)],
                        6: [lambda: ln_var(stB[0], 0)]})
            ln_sq(x, l, 4, 1, stB[0])

            # ===== per-block LN =====
            last = l == LAYERS - 1
            fin_sb = None
            if last and w_fin is not None:
                fin_sb = spool.tile([P, 2, DC], F32, tag="fin_sb")
                nc.sync.dma_start(
                    fin_sb, w_fin.rearrange("g (c p) -> p g c", p=P))
            dview = out_d.rearrange("(c p) t -> p c t", p=P) if last else None
            ln_norm(stB[0], 0, None if last else x,
                    out_stream=dview, fin_sb=fin_sb)
            ln_sums(stB[0], 1)
            ln_var(stB[0], 1)
            ln_norm(stB[0], 1, None if last else x,
                    out_stream=dview, fin_sb=fin_sb)
            # stripe-1 stat mms for blk-LN run between phases; next layer's
            # FFN1 stripe-0 mms only need x stripe 0 (already normalized)

        if LAYERS == 0:
            with tc.tile_pool(name="outp0", bufs=3) as op_:
                dview = out_d.rearrange("(c p) t -> p c t", p=P)
                for kc in range(DC):
                    for tci in range(TC):
                        sl = slice(tci * NT, (tci + 1) * NT)
                        of = op_.tile([P, NT], F32, tag="of")
                        nc.vector.tensor_copy(out=of, in_=x[:, kc, sl])
                        nc.sync.dma_start(dview[:, kc, sl], of)

    nc.finalize()
    return nc


_PROG_CACHE = {}


def _get_program(flags):
    key = tuple(sorted(flags.items())) + (LAYERS,)
    if key not in _PROG_CACHE:
        _PROG_CACHE[key] = build_program(flags)
    return _PROG_CACHE[key]


def kernel(**inputs):
    global LAST_RESULT
    f32 = lambda a: np.asarray(a, dtype=np.float32)
    bf = lambda a: np.ascontiguousarray(f32(a).astype(ml_dtypes.bfloat16))
    x = f32(inputs["x"])                       # [B, T, D]

    def triv(names_vals):
        return all(bool(np.all(f32(inputs[n]) == v)) for n, v in names_vals)

    ln_trivial = triv(
        [(f"{p}_ln_g", 1.0) for p in ("ffn1", "attn", "conv", "ffn2", "blk")]
        + [(f"{p}_ln_b", 0.0) for p in ("ffn1", "attn", "conv", "ffn2", "blk")])
    final_trivial = triv([("final_ln_g", 1.0), ("final_ln_b", 0.0)])
    bias_trivial = triv([(n, 0.0) for n in (
        "ffn1_b1", "ffn1_b2", "qkv_b", "outp_b", "pw1_b", "pw2_b",
        "ffn2_b1", "ffn2_b2")])
    dwb_trivial = triv([("dw_b", 0.0)])
    flags = dict(ln_trivial=ln_trivial, final_trivial=final_trivial,
                 bias_trivial=bias_trivial, dwb_trivial=dwb_trivial)

    nc = _get_program(flags)

    qkv = f32(inputs["qkv_w"])                # [L, D, 3D]
    dw = f32(inputs["dw_w"]).reshape(L, EC, KK) * 0.5
    dw = dw.reshape(L, CC, P, KK).transpose(0, 2, 1, 3)  # [L, P, CC, K]
    gn_aff = np.stack([f32(inputs["gn_g"]), f32(inputs["gn_b"])], axis=1)

    common = {
        "f1w1": bf(inputs["ffn1_w1"]),
        "f1w2": bf(f32(inputs["ffn1_w2"]) * 0.5),
        "f2w1": bf(inputs["ffn2_w1"]),
        "f2w2": bf(f32(inputs["ffn2_w2"]) * 0.5),
        "wq": bf(qkv[:, :, 0:D] * (DH ** -0.5)),
        "wk": bf(qkv[:, :, D : 2 * D]),
        "wv": bf(qkv[:, :, 2 * D : 3 * D]),
        "wo": bf(inputs["outp_w"]),
        "pw1": bf(inputs["pw1_w"]),
        "pw2": bf(inputs["pw2_w"]),
        "dw": np.ascontiguousarray(dw.astype(np.float32)),
        "gn_aff": np.ascontiguousarray(gn_aff.astype(np.float32)),
    }
    if not ln_trivial:
        rows = []
        for pfx in ("ffn1", "attn", "conv", "ffn2", "blk"):
            rows.append(f32(inputs[f"{pfx}_ln_g"]))
            rows.append(f32(inputs[f"{pfx}_ln_b"]))
        common["ln_gains"] = np.ascontiguousarray(
            np.stack(rows, axis=1).astype(np.float32))  # [L, 10, D]
    if not final_trivial:
        common["final_aff"] = np.ascontiguousarray(np.stack(
            [f32(inputs["final_ln_g"]), f32(inputs["final_ln_b"])]).astype(np.float32))
    if not bias_trivial:
        bias = np.zeros((L, 10, 2 * EC), np.float32)
        qb = f32(inputs["qkv_b"])
        bias[:, BR_F1B1, :FF] = f32(inputs["ffn1_b1"])
        bias[:, BR_F1B2, :D] = f32(inputs["ffn1_b2"]) * 0.5
        bias[:, BR_Q, :D] = qb[:, 0:D] * (DH ** -0.5)
        bias[:, BR_K, :D] = qb[:, D : 2 * D]
        bias[:, BR_V, :D] = qb[:, 2 * D : 3 * D]
        bias[:, BR_O, :D] = f32(inputs["outp_b"])
        bias[:, BR_P1, : 2 * EC] = f32(inputs["pw1_b"])
        bias[:, BR_P2, :D] = f32(inputs["pw2_b"])
        bias[:, BR_F2B1, :FF] = f32(inputs["ffn2_b1"])
        bias[:, BR_F2B2, :D] = f32(inputs["ffn2_b2"]) * 0.5
        common["biases"] = bf(bias)
    if not dwb_trivial:
        dwb = f32(inputs["dw_b"]).reshape(L, CC, P).transpose(0, 2, 1)
        common["dwb"] = np.ascontiguousarray(dwb.astype(np.float32))

    in_maps = []
    for c in range(B):
        m = dict(common)
        m["x_t"] = np.ascontiguousarray(x[c].T)   # [D, T] fp32
        in_maps.append(m)

    res = run_bass_kernel_spmd(
        nc, in_maps, core_ids=list(range(B)), trace=TRACE, **TRACE_KW)
    LAST_RESULT = res
    out = np.stack([r["out_t"].T for r in res.results]).astype(np.float32)
    return out


if __name__ == "__main__":
    rng = np.random.default_rng(0)
    ins = {"x": rng.standard_normal((B, T, D), dtype=np.float32)}
    print("use test.py")



# revision 20
# speedup vs baseline: 1.4119x; 1.4119x over previous
"""ConformerDecoder Trainium2 Bass kernel (stripe + software-pipelined).

Sharding: pure data-parallel over batch B=8 -> one sample per NeuronCore.

Activations live transposed [feature-on-partitions, T-free]; all matmuls use
natural-layout bf16 weights as lhsT with fp32 PSUM accumulation.

Structure vs the v1 kernel:
- Every LayerNorm is stripe-pipelined (T split in two 512-token stripes) as
  before, with stat matmuls injected into neighboring matmul stretches.
- Producer->consumer chains inside each module are SOFTWARE-PIPELINED by one
  step so the PE queue never stalls on ACT/DVE latency: FFN emits w2-acc
  matmuls for chunk m-1 after w1 matmuls of chunk m (hiding the SiLU evac);
  conv emits the depthwise-conv taps of chunk m-1 after the pw1/GLU of chunk
  m; pw2 lags its accumulation one chunk behind the GN-affine SiLU; attention
  runs (qb, hp) units in a two-stage pipeline: scores/exp/mask of unit i
  overlap AV+denominator matmuls of unit i-1.
- Depthwise-conv diagonal strips are precomputed on the host and DMA'd per
  (layer, chunk) instead of being built on DVE.
- LayerNorm subtract writes run on the Pool (gpsimd) engine (SBUF-only ops)
  to relieve DVE; psum-reading ops stay on DVE/ACT (gpsimd has no PSUM
  access).
- Optional fp8-e4m3 DoubleRow mode for the FFN second matmul (CONF_FP8W2):
  SiLU writes hb pairs straight to fp8, W2 is host-quantized fp8 * 2^6, and
  each DoubleRow matmul covers two 128-deep contraction tiles at 0.5
  cycles/row.
"""

import os
import sys
from contextlib import ExitStack

for _p in ("/opt/trn_rl_repo",):
    if _p not in sys.path:
        sys.path.insert(0, _p)

import numpy as np
import ml_dtypes

import concourse.bass as bass
import concourse.tile as tile
from concourse import bacc
from concourse import mybir
from concourse.bass_utils import run_bass_kernel_spmd

BF16 = mybir.dt.bfloat16
F8 = mybir.dt.float8e4
F32 = mybir.dt.float32
AF = mybir.ActivationFunctionType
OP = mybir.AluOpType
DRMODE = mybir.MatmulPerfMode.DoubleRow
NPF8 = ml_dtypes.float8_e4m3

L, D, H, T, B = 4, 512, 8, 1024, 8
FF = 4 * D            # 2048
EC = 2 * D            # 1024 conv channels
KK = 31               # conv kernel size
WIN = 64              # attention window
DH = D // H           # 64
P = 128
DC = D // P           # 4 feature chunks
FC = FF // P          # 16
CC = EC // P          # 8
TB = T // P           # 8 token blocks
NT = 512              # matmul moving free dim
TC = T // NT          # 2 t-stripes
KW = 3 * P            # 384: attention window width
EPS = 1e-5

SW = 6                # fp8 weight scale exponent (w * 2^6)
RSW = 2.0 ** -SW

TRACE = False          # set by test.py for profiling runs
TRACE_KW = {}
LAST_RESULT = None     # BassKernelResults of last run (read by test.py)
LAYERS = int(os.environ.get("CONF_LAYERS", str(L)))
FP8W2 = bool(int(os.environ.get("CONF_FP8W2", "1")))

# bias row indices in the packed bias tensor
BR_F1B1, BR_F1B2, BR_Q, BR_K, BR_V, BR_O, BR_P1, BR_P2, BR_F2B1, BR_F2B2 = range(10)


def _band_masks_t():
    """Transposed band masks [P, 3, P] bf16: maskT[k, c, q] = valid(q, c*128+k)."""
    q = np.arange(P)[:, None]
    c = np.arange(KW)[None, :]
    band = np.abs(q + P - c) <= WIN // 2          # [q, kw]
    q0 = band & (c >= P)
    q7 = band & (c <= 255)

    def to(m):
        mt = m.T.reshape(3, P, P).transpose(1, 0, 2)   # [k, c, q]
        return np.ascontiguousarray(mt.astype(ml_dtypes.bfloat16))

    return to(band), to(q0), to(q7)


def build_program(flags):
    ln_gen = not flags["ln_trivial"]
    bias_gen = not flags["bias_trivial"]
    fin_gen = not flags["final_trivial"]
    dwb_gen = not flags["dwb_trivial"]

    nc = bacc.Bacc("TRN2", target_bir_lowering=False, debug=False)

    xt_d = nc.dram_tensor("x_t", [D, T], F32, kind="ExternalInput").ap()
    out_d = nc.dram_tensor("out_t", [D, T], F32, kind="ExternalOutput").ap()

    def win(name, shape, dt_=BF16):
        return nc.dram_tensor(name, shape, dt_, kind="ExternalInput").ap()

    w_f1a = win("f1w1", [L, D, FF])
    w_f1b = win("f1w2", [L, FF, D], F8 if FP8W2 else BF16)
    w_f2a = win("f2w1", [L, D, FF])
    w_f2b = win("f2w2", [L, FF, D], F8 if FP8W2 else BF16)
    w_q = win("wq", [L, D, D])
    w_kk = win("wk", [L, D, D])
    w_v = win("wv", [L, D, D])
    w_o = win("wo", [L, D, D])
    w_p1 = win("pw1", [L, D, 2 * EC])
    w_p2 = win("pw2", [L, EC, D])
    w_strip = win("dwstrip", [L, CC, P, KK, P])
    w_gn = nc.dram_tensor("gn_aff", [L, 2, EC], F32, kind="ExternalInput").ap()
    w_gains = (nc.dram_tensor("ln_gains", [L, 10, D], F32, kind="ExternalInput").ap()
                if ln_gen else None)
    w_fin = (nc.dram_tensor("final_aff", [2, D], F32, kind="ExternalInput").ap()
             if fin_gen else None)
    w_bias = win("biases", [L, 10, 2 * EC]) if bias_gen else None
    w_dwb = (nc.dram_tensor("dwb", [L, P, CC], F32, kind="ExternalInput").ap()
             if dwb_gen else None)

    mt_mid, mt_q0, mt_q7 = _band_masks_t()
    bmid_d = nc.inline_tensor(mt_mid, "bandt_mid").ap()
    bq0_d = nc.inline_tensor(mt_q0, "bandt_q0").ap()
    bq7_d = nc.inline_tensor(mt_q7, "bandt_q7").ap()
    ones_d = nc.inline_tensor(np.ones((P, NT), dtype=ml_dtypes.bfloat16), "ones").ap()

    with tile.TileContext(nc) as tc, ExitStack() as ctx:
        pers = ctx.enter_context(tc.tile_pool(name="pers", bufs=1))
        wpool = ctx.enter_context(tc.tile_pool(name="w", bufs=1))
        spool = ctx.enter_context(tc.tile_pool(name="stat", bufs=1))
        hpool = ctx.enter_context(tc.tile_pool(name="h1", bufs=4))
        tpool = ctx.enter_context(tc.tile_pool(name="tanh", bufs=3))
        atpool = ctx.enter_context(tc.tile_pool(name="attn", bufs=6))
        smpool = ctx.enter_context(tc.tile_pool(name="small", bufs=8))
        cpool = ctx.enter_context(tc.tile_pool(name="conv", bufs=2))
        strpool = ctx.enter_context(tc.tile_pool(name="strip", bufs=2))
        opool = ctx.enter_context(tc.tile_pool(name="outp", bufs=2))
        psum = ctx.enter_context(tc.tile_pool(name="ps", bufs=8, space="PSUM"))

        ones = pers.tile([P, NT], BF16, tag="ones")
        nc.sync.dma_start(ones, ones_d)
        bands = {}
        for nm, dd in (("mid", bmid_d), ("q0", bq0_d), ("q7", bq7_d)):
            bt = pers.tile([P, 3, P], BF16, tag=f"band_{nm}")
            nc.sync.dma_start(bt, dd)
            bands[nm] = bt

        x = pers.tile([P, DC, T], BF16, tag="x")
        with tc.tile_pool(name="xin", bufs=1) as xinp:
            for kc in range(DC):
                xf = xinp.tile([P, T], F32, name=f"xf{kc}", tag="xf")
                nc.sync.dma_start(
                    xf, xt_d.rearrange("(c p) t -> c p t", p=P)[kc])
                nc.scalar.copy(out=x[:, kc], in_=xf)

        xh = pers.tile([P, DC, T], BF16, tag="xhat")
        qT = pers.tile([P, DC, T], BF16, tag="qT")
        kT = pers.tile([P, DC, T + 2 * P], BF16, tag="kT")
        vN = pers.tile([P, TB + 2, D], BF16, tag="vN")
        oT = pers.tile([P, DC, T], BF16, tag="oT")
        c2 = pers.tile([P, CC, T], BF16, tag="c2")
        nc.vector.memset(kT[:, :, 0:P], 0.0)
        nc.vector.memset(kT[:, :, P + T :], 0.0)
        nc.vector.memset(vN[:, 0, :], 0.0)
        nc.vector.memset(vN[:, TB + 1, :], 0.0)

        # ---------------- LayerNorm (stripe-pipelined) ----------------

        def ln_rstd(var, out_r, niter=2):
            r = spool.tile(list(var.shape), F32, tag="newt_r", bufs=2)
            nc.vector.tensor_scalar(
                out=r, in0=var, scalar1=-0.5, scalar2=1.5,
                op0=OP.mult, op1=OP.add)
            t1 = spool.tile(list(var.shape), F32, tag="newt_t", bufs=2)
            for it in range(niter):
                nc.vector.tensor_tensor(t1, r, r, OP.mult)
                nc.vector.scalar_tensor_tensor(
                    out=t1, in0=t1, scalar=-0.5, in1=var, op0=OP.mult, op1=OP.mult)
                dst = out_r if it == niter - 1 else r
                nc.vector.scalar_tensor_tensor(
                    out=dst, in0=t1, scalar=1.5, in1=r, op0=OP.add, op1=OP.mult)

        class LNState:
            __slots__ = ("src", "lidx", "which", "ps", "r_bf", "mr_bf", "x2")

        def ln_sq(src, lidx, which, tci, st=None):
            """ACT squares for stripe tci; allocates state on first call."""
            if st is None:
                st = LNState()
                st.src, st.lidx, st.which = src, lidx, which
                st.ps = [None, None]
                st.r_bf = spool.tile([P, T], BF16, tag="r_bf", bufs=3)
                st.mr_bf = spool.tile([P, T], BF16, tag="mr_bf", bufs=3)
                st.x2 = [None, None]
            sl = slice(tci * NT, (tci + 1) * NT)
            x2 = spool.tile([P, DC, NT], BF16, tag="ln_x2", bufs=1)
            nc.scalar.activation(out=x2, in_=st.src[:, :, sl], func=AF.Square)
            st.x2[tci] = x2
            return st

        def ln_sums(st, tci):
            """PE ones-matmul sums for stripe tci (inject into a mm stretch)."""
            sl = slice(tci * NT, (tci + 1) * NT)
            ps_s = psum.tile([P, NT], F32, tag="mm", name="ps_s")
            ps_q = psum.tile([P, NT], F32, tag="mm", name="ps_q")
            for kc in range(DC):
                nc.tensor.matmul(ps_s, lhsT=ones[:, 0:P], rhs=st.src[:, kc, sl],
                                 start=(kc == 0), stop=(kc == DC - 1))
            for kc in range(DC):
                nc.tensor.matmul(ps_q, lhsT=ones[:, 0:P], rhs=st.x2[tci][:, kc],
                                 start=(kc == 0), stop=(kc == DC - 1))
            st.ps[tci] = (ps_s, ps_q)

        def ln_var(st, tci):
            """msq/mean on ACT, var + Newton rsqrt + mr on DVE."""
            sl = slice(tci * NT, (tci + 1) * NT)
            ps_s, ps_q = st.ps[tci]
            msq = spool.tile([P, NT], F32, tag="msq", bufs=1)
            nc.scalar.activation(out=msq, in_=ps_s, func=AF.Square, scale=1.0 / D)
            mean = spool.tile([P, NT], F32, tag="mean", bufs=1)
            nc.scalar.mul(out=mean, in_=ps_s, mul=1.0 / D)
            var = spool.tile([P, NT], F32, tag="var", bufs=2)
            nc.vector.scalar_tensor_tensor(
                out=var, in0=ps_q, scalar=1.0 / D, in1=msq,
                op0=OP.mult, op1=OP.subtract)
            ln_rstd(var, st.r_bf[:, sl])
            nc.vector.tensor_tensor(st.mr_bf[:, sl], mean, st.r_bf[:, sl], OP.mult)

        def ln_norm(st, tci, dst, out_stream=None, fin_sb=None):
            """normalize stripe tci of src into dst; subtract on Pool."""
            sl = slice(tci * NT, (tci + 1) * NT)
            g_sb = None
            if w_gains is not None:
                g_sb = spool.tile([P, 2, DC], F32, tag="g_sb", bufs=2)
                nc.sync.dma_start(
                    g_sb, w_gains[st.lidx, 2 * st.which : 2 * st.which + 2]
                    .rearrange("g (c p) -> p g c", p=P))
            for kc in range(DC):
                u = tpool.tile([P, NT], BF16, tag="ln_u")
                nc.vector.tensor_tensor(u, st.src[:, kc, sl], st.r_bf[:, sl],
                                        OP.mult)
                if dst is not None:
                    tgt = dst[:, kc, sl]
                    nc.gpsimd.tensor_tensor(tgt, u, st.mr_bf[:, sl], OP.subtract)
                    if g_sb is not None:
                        nc.scalar.activation(
                            out=tgt, in_=tgt, func=AF.Identity,
                            bias=g_sb[:, 1, kc : kc + 1], scale=g_sb[:, 0, kc : kc + 1])
                if out_stream is not None:
                    dview = out_stream
                    of = opool.tile([P, NT], F32, tag="of")
                    nc.vector.tensor_tensor(of, u, st.mr_bf[:, sl], OP.subtract)
                    if g_sb is not None:
                        nc.scalar.activation(
                            out=of, in_=of, func=AF.Identity,
                            bias=g_sb[:, 1, kc : kc + 1], scale=g_sb[:, 0, kc : kc + 1])
                    if fin_sb is not None:
                        nc.scalar.activation(
                            out=of, in_=of, func=AF.Identity,
                            bias=fin_sb[:, 1, kc : kc + 1], scale=fin_sb[:, 0, kc : kc + 1])
                    nc.sync.dma_start(dview[:, kc, sl], of)

        def load_w(dram, lidx, tag, dt_=BF16):
            _, fin, fout = dram.shape
            wt = wpool.tile([P, fin // P, fout], dt_, tag=tag)
            nc.sync.dma_start(wt, dram[lidx].rearrange("(c p) f -> p c f", p=P))
            return wt

        bias_sb = [None]

        def bias_mm(ps, row, mslice, tcslice_n):
            if bias_sb[0] is None:
                return
            nc.tensor.matmul(
                ps, lhsT=bias_sb[0][0:1, row, mslice], rhs=ones[0:1, 0:tcslice_n],
                start=False, stop=True, skip_group_check=True)

        def run_inject(inject, idx):
            if inject and idx in inject:
                for fn in inject[idx]:
                    fn()

        def emit_ffn_stripe(w1, w2, rows, src, tci, inject=None):
            """One FFN stripe; w2-acc matmuls lag one m-chunk behind SiLU."""
            sl = slice(tci * NT, (tci + 1) * NT)
            acc = [psum.tile([P, NT], F32, tag="mm", name=f"acc{i}")
                   for i in range(DC)]
            pend = []          # chunks whose silu is emitted but w2-acc isn't

            def w2acc_bf16(m, hb):
                for dcc in range(DC):
                    nc.tensor.matmul(
                        acc[dcc], lhsT=w2[:, m, dcc * P : (dcc + 1) * P], rhs=hb,
                        start=(m == 0), stop=(m == FC - 1 and not bias_gen),
                        skip_group_check=True)

            def w2acc_dr(mp, hb2):
                for dcc in range(DC):
                    nc.tensor.matmul(
                        acc[dcc], lhsT=w2[:, 2 * mp : 2 * mp + 2,
                                          dcc * P : (dcc + 1) * P],
                        rhs=hb2[:, 0:2, :],
                        start=(mp == 0), stop=(mp == FC // 2 - 1 and not bias_gen),
                        perf_mode=DRMODE, skip_group_check=True)

            hb2 = None
            for m in range(FC):
                run_inject(inject, m)
                ph = psum.tile([P, NT], F32, tag="mm", name="ph")
                for kc in range(DC):
                    nc.tensor.matmul(
                        ph, lhsT=w1[:, kc, m * P : (m + 1) * P], rhs=src[:, kc, sl],
                        start=(kc == 0), stop=(kc == DC - 1 and not bias_gen))
                bias_mm(ph, rows[0], slice(m * P, (m + 1) * P), NT)
                if FP8W2:
                    if m % 2 == 0:
                        hb2 = hpool.tile([P, 2, NT], F8, tag="h8")
                    nc.scalar.activation(out=hb2[:, m % 2], in_=ph, func=AF.Silu)
                    if m % 2 == 1:
                        pend.append((m // 2, hb2))
                else:
                    hb = hpool.tile([P, NT], BF16, tag="h1")
                    nc.scalar.activation(out=hb, in_=ph, func=AF.Silu)
                    pend.append((m, hb))
                if len(pend) > 1:
                    mm, hh = pend.pop(0)
                    (w2acc_dr if FP8W2 else w2acc_bf16)(mm, hh)
            run_inject(inject, FC)
            for mm, hh in pend:
                (w2acc_dr if FP8W2 else w2acc_bf16)(mm, hh)
            for dcc in range(DC):
                bias_mm(acc[dcc], rows[1], slice(dcc * P, (dcc + 1) * P), NT)
                nc.vector.scalar_tensor_tensor(
                    out=x[:, dcc, sl], in0=acc[dcc],
                    scalar=RSW if FP8W2 else 1.0, in1=x[:, dcc, sl],
                    op0=OP.mult if FP8W2 else OP.bypass, op1=OP.add)

        # ---------------- attention sub-emitters ----------------

        def emit_qk_stripe(wq, wk, tci, inject=None):
            sl = slice(tci * NT, (tci + 1) * NT)
            for m in range(DC):
                run_inject(inject, m)
                pq = psum.tile([P, NT], F32, tag="mm", name="pq")
                for kc in range(DC):
                    nc.tensor.matmul(
                        pq, lhsT=wq[:, kc, m * P : (m + 1) * P], rhs=xh[:, kc, sl],
                        start=(kc == 0), stop=(kc == DC - 1 and not bias_gen))
                bias_mm(pq, BR_Q, slice(m * P, (m + 1) * P), NT)
                nc.scalar.copy(out=qT[:, m, sl], in_=pq)
                pk = psum.tile([P, NT], F32, tag="mm", name="pk")
                for kc in range(DC):
                    nc.tensor.matmul(
                        pk, lhsT=wk[:, kc, m * P : (m + 1) * P], rhs=xh[:, kc, sl],
                        start=(kc == 0), stop=(kc == DC - 1 and not bias_gen))
                bias_mm(pk, BR_K, slice(m * P, (m + 1) * P), NT)
                nc.scalar.copy(
                    out=kT[:, m, P + tci * NT : P + (tci + 1) * NT], in_=pk)

        def emit_v_blocks(wv, tbs, inject=None):
            for i, tb in enumerate(tbs):
                run_inject(inject, i)
                pv = psum.tile([P, NT], F32, tag="mm", name="pv")
                for kc in range(DC):
                    nc.tensor.matmul(
                        pv, lhsT=xh[:, kc, tb * P : (tb + 1) * P],
                        rhs=wv[:, kc, 0:D],
                        start=(kc == 0), stop=(kc == DC - 1 and not bias_gen))
                if bias_gen:
                    nc.tensor.matmul(
                        pv, lhsT=ones[0:1, 0:P], rhs=bias_sb[0][0:1, BR_V, 0:D],
                        start=False, stop=True, skip_group_check=True)
                nc.scalar.copy(out=vN[:, tb + 1, :], in_=pv)

        def attn_unit_S(qb, hp):
            """Scores + exp + mask for one (qb, hp) unit (both hh)."""
            band = bands["q0"] if qb == 0 else (
                bands["q7"] if qb == TB - 1 else bands["mid"])
            ats = []
            for hh in range(2):
                pr = slice(hh * DH, (hh + 1) * DH)
                st = psum.tile([P, 3, P], F32, tag="mm", name="st")
                for c in range(3):
                    nc.tensor.matmul(
                        st[:, c, :],
                        lhsT=kT[pr, hp, qb * P + c * P : qb * P + (c + 1) * P],
                        rhs=qT[pr, hp, qb * P : (qb + 1) * P],
                        start=True, stop=True, skip_group_check=True)
                at = atpool.tile([P, 3, P], BF16, tag="at")
                nc.scalar.activation(out=at, in_=st, func=AF.Exp)
                nc.vector.tensor_tensor(at, at, band, OP.mult)
                ats.append(at)
            return (qb, hp, ats)

        def attn_unit_A(unit):
            """AV + denominator matmuls + softmax divide for one unit."""
            qb, hp, ats = unit
            po = psum.tile([P, P], F32, tag="mm", name="po")
            den = psum.tile([P, P], F32, tag="mm", name="den")
            for hh in range(2):
                pr = slice(hh * DH, (hh + 1) * DH)
                at = ats[hh]
                h = hp * 2 + hh
                hc = slice(h * DH, (h + 1) * DH)
                for c in range(3):
                    nc.tensor.matmul(
                        po[pr, :], lhsT=vN[:, qb + c, hc], rhs=at[:, c, :],
                        start=(c == 0), stop=(c == 2), skip_group_check=True)
                for c in range(3):
                    nc.tensor.matmul(
                        den[pr, :], lhsT=ones[:, 0:DH], rhs=at[:, c, :],
                        start=(c == 0), stop=(c == 2), skip_group_check=True)
            rden = atpool.tile([P, P], F32, tag="rden", bufs=2)
            nc.vector.reciprocal(out=rden, in_=den)
            nc.vector.tensor_tensor(
                oT[:, hp, qb * P : (qb + 1) * P], po, rden, OP.mult)

        def emit_outproj_stripe(wo, tci, inject=None):
            sl = slice(tci * NT, (tci + 1) * NT)
            for m in range(DC):
                run_inject(inject, m)
                pp = psum.tile([P, NT], F32, tag="mm", name="pp")
                for kc in range(DC):
                    nc.tensor.matmul(
                        pp, lhsT=wo[:, kc, m * P : (m + 1) * P], rhs=oT[:, kc, sl],
                        start=(kc == 0), stop=(kc == DC - 1 and not bias_gen))
                bias_mm(pp, BR_O, slice(m * P, (m + 1) * P), NT)
                nc.vector.scalar_tensor_tensor(
                    out=x[:, m, sl], in0=pp, scalar=1.0, in1=x[:, m, sl],
                    op0=OP.bypass, op1=OP.add)

        # ---------------- conv sub-emitters ----------------

        def emit_conv(l, p1, dwb_sb, gn_ps, inject=None):
            """pw1+GLU, with the dw-conv taps of chunk m-1 pipelined behind
            the pw1 matmuls of chunk m; GN sums accumulate in the dw block."""
            gs, gq = gn_ps
            pend = []

            def dw_block(m, strip, cp, co):
                for tci in range(TC):
                    pc = psum.tile([P, NT], F32, tag="mm", name="pc")
                    for kk in range(KK):
                        rhs = (cp[:, kk + tci * NT : kk + tci * NT + NT]
                               if kk % 2 == 0 else
                               co[:, kk - 1 + tci * NT : kk - 1 + tci * NT + NT])
                        nc.tensor.matmul(pc, lhsT=strip[:, kk, :], rhs=rhs,
                                         start=(kk == 0), stop=(kk == KK - 1),
                                         skip_group_check=True)
                    csl = c2[:, m, tci * NT : (tci + 1) * NT]
                    if dwb_sb is not None:
                        nc.scalar.activation(out=csl, in_=pc, func=AF.Identity,
                                             bias=dwb_sb[:, m : m + 1])
                    else:
                        nc.scalar.copy(out=csl, in_=pc)
                c2sq = spool.tile([P, T], BF16, tag="gnsq", bufs=2)
                nc.scalar.activation(out=c2sq, in_=c2[:, m], func=AF.Square)
                for tci in range(TC):
                    sl = slice(tci * NT, (tci + 1) * NT)
                    nc.tensor.matmul(gs, lhsT=ones[:, 0:P], rhs=c2[:, m, sl],
                                     start=(m == 0 and tci == 0),
                                     stop=(m == CC - 1 and tci == TC - 1),
                                     skip_group_check=True)
                    nc.tensor.matmul(gq, lhsT=ones[:, 0:P], rhs=c2sq[:, sl],
                                     start=(m == 0 and tci == 0),
                                     stop=(m == CC - 1 and tci == TC - 1),
                                     skip_group_check=True)

            for m in range(CC):
                run_inject(inject, m)
                strip = strpool.tile([P, KK, P], BF16, tag="strip")
                nc.sync.dma_start(strip, w_strip[l, m])
                cp = cpool.tile([P, KK - 1 + T + 1], BF16, tag="cp")
                co = cpool.tile([P, KK - 1 + T + 1], BF16, tag="co")
                nc.vector.memset(cp[:, 0 : KK // 2], 0.0)
                nc.vector.memset(cp[:, KK // 2 + T :], 0.0)
                for tci in range(TC):
                    sl = slice(tci * NT, (tci + 1) * NT)
                    pb = psum.tile([P, NT], F32, tag="mm", name="pb")
                    for kc in range(DC):
                        nc.tensor.matmul(
                            pb, lhsT=p1[:, kc, EC + m * P : EC + (m + 1) * P],
                            rhs=xh[:, kc, sl],
                            start=(kc == 0), stop=(kc == DC - 1 and not bias_gen))
                    bias_mm(pb, BR_P1, slice(EC + m * P, EC + (m + 1) * P), NT)
                    tb_ = tpool.tile([P, NT], BF16, tag="th")
                    nc.scalar.activation(out=tb_, in_=pb, func=AF.Tanh, scale=0.5)
                    pa = psum.tile([P, NT], F32, tag="mm", name="pa")
                    for kc in range(DC):
                        nc.tensor.matmul(
                            pa, lhsT=p1[:, kc, m * P : (m + 1) * P],
                            rhs=xh[:, kc, sl],
                            start=(kc == 0), stop=(kc == DC - 1 and not bias_gen))
                    bias_mm(pa, BR_P1, slice(m * P, (m + 1) * P), NT)
                    nc.vector.scalar_tensor_tensor(
                        out=cp[:, KK // 2 + tci * NT : KK // 2 + (tci + 1) * NT],
                        in0=tb_, scalar=1.0, in1=pa, op0=OP.add, op1=OP.mult)
                nc.gpsimd.tensor_copy(out=co[:, 0 : KK - 1 + T],
                                      in_=cp[:, 1 : KK + T])
                pend.append((m, strip, cp, co))
                if len(pend) > 1:
                    dw_block(*pend.pop(0))
            for args in pend:
                dw_block(*args)

        def emit_gn_finalize(l, gn_ps):
            """GroupNorm scalar chain -> per-channel affine (a_t, b_t)."""
            gs_ps, gq_ps = gn_ps
            rs = smpool.tile([P, 1], F32, tag="gs")
            rq = smpool.tile([P, 1], F32, tag="gq")
            nc.vector.tensor_reduce(out=rs, in_=gs_ps,
                                    axis=mybir.AxisListType.X, op=OP.add)
            nc.vector.tensor_reduce(out=rq, in_=gq_ps,
                                    axis=mybir.AxisListType.X, op=OP.add)
            mg = smpool.tile([P, 1], F32, tag="mg")
            nc.vector.tensor_scalar_mul(out=mg, in0=rs, scalar1=1.0 / (EC * T))
            msqg = smpool.tile([P, 1], F32, tag="msqg")
            nc.vector.tensor_tensor(msqg, mg, mg, OP.mult)
            varg = smpool.tile([P, 1], F32, tag="varg")
            nc.vector.scalar_tensor_tensor(
                out=varg, in0=rq, scalar=1.0 / (EC * T), in1=msqg,
                op0=OP.mult, op1=OP.subtract)
            nc.vector.tensor_scalar_add(out=varg, in0=varg, scalar1=EPS)
            rg = smpool.tile([P, 1], F32, tag="rg")
            ln_rstd(varg, rg, niter=14)
            gaff = spool.tile([P, 2, CC], F32, tag="gaff", bufs=2)
            nc.sync.dma_start(gaff, w_gn[l].rearrange("g (c p) -> p g c", p=P))
            a_t = spool.tile([P, CC], F32, tag="a_t", bufs=2)
            nc.vector.tensor_scalar_mul(out=a_t, in0=gaff[:, 0], scalar1=rg)
            mneg = smpool.tile([P, 1], F32, tag="mneg")
            nc.vector.tensor_scalar_mul(out=mneg, in0=mg, scalar1=-1.0)
            b_t = spool.tile([P, CC], F32, tag="b_t", bufs=2)
            nc.vector.scalar_tensor_tensor(
                out=b_t, in0=a_t, scalar=mneg, in1=gaff[:, 1],
                op0=OP.mult, op1=OP.add)
            return a_t, b_t

        def emit_pw2(p2, a_t, b_t, tci, inject=None):
            """GN affine + SiLU on ACT per chunk; m-outer pw2 acc lags one."""
            sl = slice(tci * NT, (tci + 1) * NT)
            acc = [psum.tile([P, NT], F32, tag="mm", name=f"cacc{i}")
                   for i in range(DC)]
            pend = []

            def accm(m):
                for dcc in range(DC):
                    nc.tensor.matmul(
                        acc[dcc], lhsT=p2[:, m, dcc * P : (dcc + 1) * P],
                        rhs=c2[:, m, sl],
                        start=(m == 0), stop=(m == CC - 1 and not bias_gen),
                        skip_group_check=True)

            for m in range(CC):
                run_inject(inject, m)
                if tci == 0:
                    nc.scalar.activation(
                        out=c2[:, m], in_=c2[:, m], func=AF.Silu,
                        bias=b_t[:, m : m + 1], scale=a_t[:, m : m + 1])
                    pend.append(m)
                    if len(pend) > 1:
                        accm(pend.pop(0))
                else:
                    accm(m)
            run_inject(inject, CC)
            for m in pend:
                accm(m)
            for dcc in range(DC):
                bias_mm(acc[dcc], BR_P2, slice(dcc * P, (dcc + 1) * P), NT)
                nc.vector.scalar_tensor_tensor(
                    out=x[:, dcc, sl], in0=acc[dcc], scalar=1.0,
                    in1=x[:, dcc, sl], op0=OP.bypass, op1=OP.add)

        # ================= layer driver =================

        # initial LN for layer 0 (x raw -> xh)
        st0 = ln_sq(x, 0, 0, 0)
        ln_sums(st0, 0)
        ln_var(st0, 0)
        ln_sq(x, 0, 0, 1, st0)
        ln_sums(st0, 1)
        ln_var(st0, 1)
        ln_norm(st0, 0, xh)
        ln_norm(st0, 1, xh)

        for l in range(LAYERS):
            if bias_gen:
                bt = wpool.tile([1, 10, 2 * EC], BF16, tag="bias")
                nc.sync.dma_start(bt, w_bias[l])
                bias_sb[0] = bt

            w1 = load_w(w_f1a, l, "w1")
            w2 = load_w(w_f1b, l, "w2", F8 if FP8W2 else BF16)
            src1 = xh if l == 0 else x

            # ===== FFN1 (+ attn-LN pipelined) =====
            stA = [None]
            emit_ffn_stripe(w1, w2, (BR_F1B1, BR_F1B2), src1, 0)
            stA[0] = ln_sq(x, l, 1, 0)
            emit_ffn_stripe(
                w1, w2, (BR_F1B1, BR_F1B2), src1, 1,
                inject={4: [lambda: ln_sums(stA[0], 0)],
                        6: [lambda: ln_var(stA[0], 0)],
                        10: [lambda: ln_norm(stA[0], 0, xh)]})
            ln_sq(x, l, 1, 1, stA[0])

            # ===== attention =====
            wq = load_w(w_q, l, "wq")
            wk = load_w(w_kk, l, "wk")
            wv = load_w(w_v, l, "wv")
            wo = load_w(w_o, l, "wo")
            emit_qk_stripe(wq, wk, 0,
                           inject={1: [lambda: ln_sums(stA[0], 1)],
                                   2: [lambda: ln_var(stA[0], 1)],
                                   3: [lambda: ln_norm(stA[0], 1, xh)]})
            emit_qk_stripe(wq, wk, 1)
            emit_v_blocks(wv, range(TB))
            stC = [None]

            def core_tail_0():
                emit_outproj_stripe(wo, 0)
                stC[0] = ln_sq(x, l, 2, 0)

            # two-stage (qb, hp) pipeline over all 32 units
            s_inject = {18: [lambda: ln_sums(stC[0], 0)]}
            prev_unit = None
            for ui in range(TB * DC):
                qb, hp = divmod(ui, DC)
                run_inject(s_inject, ui)
                cur = attn_unit_S(qb, hp)
                if prev_unit is not None:
                    attn_unit_A(prev_unit)
                prev_unit = cur
                if ui == 16:
                    core_tail_0()
            attn_unit_A(prev_unit)
            emit_outproj_stripe(wo, 1, inject={2: [lambda: ln_var(stC[0], 0)]})
            ln_sq(x, l, 2, 1, stC[0])
            ln_norm(stC[0], 0, xh)

            # ===== conv module =====
            p1 = load_w(w_p1, l, "w1")
            p2 = load_w(w_p2, l, "w2")
            dwb_sb = None
            if dwb_gen:
                dwb_sb = wpool.tile([P, CC], F32, tag="dwb")
                nc.sync.dma_start(dwb_sb, w_dwb[l])
            gs = psum.tile([P, NT], F32, tag="mm", name="gn_s")
            gq = psum.tile([P, NT], F32, tag="mm", name="gn_q")

            def conv_inject():
                ln_sums(stC[0], 1)
                ln_var(stC[0], 1)
                ln_norm(stC[0], 1, xh)

            emit_conv(l, p1, dwb_sb, (gs, gq), inject={0: [conv_inject]})

            a_t, b_t = emit_gn_finalize(l, (gs, gq))
            st2 = [None]
            emit_pw2(p2, a_t, b_t, 0)
            st2[0] = ln_sq(x, l, 3, 0)
            emit_pw2(p2, a_t, b_t, 1,
                     inject={2: [lambda: ln_sums(st2[0], 0)],
                             4: [lambda: ln_var(st2[0], 0)],
                             6: [lambda: ln_norm(st2[0], 0, xh)]})
            ln_sq(x, l, 3, 1, st2[0])

            # ===== FFN2 (+ blk-LN pipelined) =====
            w1b = load_w(w_f2a, l, "w1")
            w2b = load_w(w_f2b, l, "w2", F8 if FP8W2 else BF16)
            stB = [None]
            emit_ffn_stripe(
                w1b, w2b, (BR_F2B1, BR_F2B2), xh, 0,
                inject={2: [lambda: ln_sums(st2[0], 1)],
                        4: [lambda: ln_var(st2[0], 1)],
                        8: [lambda: ln_norm(st2[0], 1, xh)]})
            stB[0] = ln_sq(x, l, 4, 0)
            emit_ffn_stripe(
                w1b, w2b, (BR_F2B1, BR_F2B2), xh, 1,
                inject={4: [lambda: ln_sums(stB[0], 0)],
                        6: [lambda: ln_var(stB[0], 0)]})
            ln_sq(x, l, 4, 1, stB[0])

            # ===== per-block LN =====
            last = l == LAYERS - 1
            fin_sb = None
            if last and w_fin is not None:
                fin_sb = spool.tile([P, 2, DC], F32, tag="fin_sb")
                nc.sync.dma_start(
                    fin_sb, w_fin.rearrange("g (c p) -> p g c", p=P))
            dview = out_d.rearrange("(c p) t -> p c t", p=P) if last else None
            ln_norm(stB[0], 0, None if last else x,
                    out_stream=dview, fin_sb=fin_sb)
            ln_sums(stB[0], 1)
            ln_var(stB[0], 1)
            ln_norm(stB[0], 1, None if last else x,
                    out_stream=dview, fin_sb=fin_sb)

        if LAYERS == 0:
            with tc.tile_pool(name="outp0", bufs=3) as op_:
                dview = out_d.rearrange("(c p) t -> p c t", p=P)
                for kc in range(DC):
                    for tci in range(TC):
                        sl = slice(tci * NT, (tci + 1) * NT)
                        of = op_.tile([P, NT], F32, tag="of")
                        nc.vector.tensor_copy(out=of, in_=x[:, kc, sl])
                        nc.sync.dma_start(dview[:, kc, sl], of)

    nc.finalize()
    return nc


_PROG_CACHE = {}


def _get_program(flags):
    key = tuple(sorted(flags.items())) + (LAYERS, FP8W2)
    if key not in _PROG_CACHE:
        _PROG_CACHE[key] = build_program(flags)
    return _PROG_CACHE[key]


def kernel(**inputs):
    global LAST_RESULT
    f32 = lambda a: np.asarray(a, dtype=np.float32)
    bf = lambda a: np.ascontiguousarray(f32(a).astype(ml_dtypes.bfloat16))
    x = f32(inputs["x"])                       # [B, T, D]

    def triv(names_vals):
        return all(bool(np.all(f32(inputs[n]) == v)) for n, v in names_vals)

    ln_trivial = triv(
        [(f"{p}_ln_g", 1.0) for p in ("ffn1", "attn", "conv", "ffn2", "blk")]
        + [(f"{p}_ln_b", 0.0) for p in ("ffn1", "attn", "conv", "ffn2", "blk")])
    final_trivial = triv([("final_ln_g", 1.0), ("final_ln_b", 0.0)])
    bias_trivial = triv([(n, 0.0) for n in (
        "ffn1_b1", "ffn1_b2", "qkv_b", "outp_b", "pw1_b", "pw2_b",
        "ffn2_b1", "ffn2_b2")])
    dwb_trivial = triv([("dw_b", 0.0)])
    flags = dict(ln_trivial=ln_trivial, final_trivial=final_trivial,
                 bias_trivial=bias_trivial, dwb_trivial=dwb_trivial)

    nc = _get_program(flags)

    qkv = f32(inputs["qkv_w"])                # [L, D, 3D]
    # depthwise conv -> diagonal strips [L, CC, P, KK, P] bf16 (0.5-folded
    # for the tanh GLU trick)
    dw = f32(inputs["dw_w"]).reshape(L, EC, KK) * 0.5
    dw = dw.reshape(L, CC, P, KK)
    strips = dw[:, :, :, :, None] * np.eye(P, dtype=np.float32)[
        None, None, :, None, :]
    gn_aff = np.stack([f32(inputs["gn_g"]), f32(inputs["gn_b"])], axis=1)

    if FP8W2:
        w2q = lambda a: np.ascontiguousarray(
            (f32(a) * 0.5 * float(2 ** SW)).astype(NPF8))
    else:
        w2q = lambda a: bf(f32(a) * 0.5)

    common = {
        "f1w1": bf(inputs["ffn1_w1"]),
        "f1w2": w2q(inputs["ffn1_w2"]),
        "f2w1": bf(inputs["ffn2_w1"]),
        "f2w2": w2q(inputs["ffn2_w2"]),
        "wq": bf(qkv[:, :, 0:D] * (DH ** -0.5)),
        "wk": bf(qkv[:, :, D : 2 * D]),
        "wv": bf(qkv[:, :, 2 * D : 3 * D]),
        "wo": bf(inputs["outp_w"]),
        "pw1": bf(inputs["pw1_w"]),
        "pw2": bf(inputs["pw2_w"]),
        "dwstrip": np.ascontiguousarray(strips.astype(ml_dtypes.bfloat16)),
        "gn_aff": np.ascontiguousarray(gn_aff.astype(np.float32)),
    }
    if not ln_trivial:
        rows = []
        for pfx in ("ffn1", "attn", "conv", "ffn2", "blk"):
            rows.append(f32(inputs[f"{pfx}_ln_g"]))
            rows.append(f32(inputs[f"{pfx}_ln_b"]))
        common["ln_gains"] = np.ascontiguousarray(
            np.stack(rows, axis=1).astype(np.float32))  # [L, 10, D]
    if not final_trivial:
        common["final_aff"] = np.ascontiguousarray(np.stack(
            [f32(inputs["final_ln_g"]), f32(inputs["final_ln_b"])]).astype(np.float32))
    if not bias_trivial:
        bias = np.zeros((L, 10, 2 * EC), np.float32)
        qb = f32(inputs["qkv_b"])
        w2s = float(2 ** SW) if FP8W2 else 1.0
        bias[:, BR_F1B1, :FF] = f32(inputs["ffn1_b1"])
        bias[:, BR_F1B2, :D] = f32(inputs["ffn1_b2"]) * 0.5 * w2s
        bias[:, BR_Q, :D] = qb[:, 0:D] * (DH ** -0.5)
        bias[:, BR_K, :D] = qb[:, D : 2 * D]
        bias[:, BR_V, :D] = qb[:, 2 * D : 3 * D]
        bias[:, BR_O, :D] = f32(inputs["outp_b"])
        bias[:, BR_P1, : 2 * EC] = f32(inputs["pw1_b"])
        bias[:, BR_P2, :D] = f32(inputs["pw2_b"])
        bias[:, BR_F2B1, :FF] = f32(inputs["ffn2_b1"])
        bias[:, BR_F2B2, :D] = f32(inputs["ffn2_b2"]) * 0.5 * w2s
        common["biases"] = bf(bias)
    if not dwb_trivial:
        dwb = f32(inputs["dw_b"]).reshape(L, CC, P).transpose(0, 2, 1)
        common["dwb"] = np.ascontiguousarray(dwb.astype(np.float32))

    in_maps = []
    for c in range(B):
        m = dict(common)
        m["x_t"] = np.ascontiguousarray(x[c].T)   # [D, T] fp32
        in_maps.append(m)

    res = run_bass_kernel_spmd(
        nc, in_maps, core_ids=list(range(B)), trace=TRACE, **TRACE_KW)
    LAST_RESULT = res
    out = np.stack([r["out_t"].T for r in res.results]).astype(np.float32)
    return out


if __name__ == "__main__":
    print("use test.py")
